# revision 42
# baseline (speedup 1.0000x reference)
"""GCN block (GraphConv + BatchNorm1d + ReLU) on 8 Trainium2 NeuronCores.

Strategy: partition dst nodes across the 8 cores; every core keeps the FULL
x table (an external input, so it is staged for free) in its HBM and gathers
x[src] rows directly — the weight is applied AFTER aggregation, which is
exact because aggregation is linear:

    agg[d] = sum_e  nsrc[src_e] * ndst[d] * x[src_e]        (segmented sum)
    y_pre[d] = agg[d] @ W + b ; h = relu(y_pre) ; BN(h)

This removes the h AllGather of the original design entirely (it cost
~700us of collective time on the critical path).

Layout is feature-major on chip ([feature, node]) so bias and the BN affine
are per-partition ACT ops, and BN batch sums fall out of the ACT
accumulator for free.

The dst->core assignment is ours to choose, so nodes are assigned to
(core, group) slots by a degree-profile-aware round-robin (nodes with equal
per-bank in-degree profiles are dealt cyclically across all 784 slots).
This equalizes every (group, bank) bucket's edge count across the 8 cores,
collapsing the shared-NEFF padding slack from ~11% to ~2%; buckets are then
padded to 16-slot granularity (the dma_gather index wrap).  128-edge blocks
may straddle group boundaries: each (block, group) pair in the shared
schedule gets its own one-hot matrix M built from a per-pair pre-shifted
dst-offset column (values outside [0,128) give zero columns, so foreign
and pad slots contribute exactly 0).  Per-edge degree norms are folded into
M by the dual-op tensor_scalar (is_equal then mult) at zero extra cost.

Groups are processed in chunks of GC=3: each group owns a dedicated
PSUM-bank pair whose accumulation chain spans all 4 src banks of its chunk
(6 seg banks + 2 W banks = all 8).  On group completion: agg pair -> SBUF
bf16, W matmul, relu(+bias ptr) with ACT accumulator emitting BN sums,
Square pass emitting sum-of-squares; AllReduce [128,2]; per-partition
affine; y^T written to HBM (host permutes rows back).

Host-side work is limited to integer index bookkeeping and layout/dtype
transforms. All floating-point math runs on device.
"""
import math
import os
import sys

sys.path.insert(0, "/opt/trn_rl_repo")

import numpy as np

import concourse.bacc as bacc
import concourse.bass as bass
import concourse.mybir as mybir
import concourse.tile as tile
from concourse import bass_utils

F32 = mybir.dt.float32
BF16 = mybir.dt.bfloat16
I16 = mybir.dt.int16

CFG = dict(
    N=100000,
    E=1600000,
    IN=256,
    OUT=128,
    NCORES=8,
    GRP=128,          # dst nodes per segment group (= one-hot free dim)
    NBANKS=4,         # src banks (bank rows must stay < 32768 for int16 idx)
    XB=25000,         # rows per x bank
    GC=3,             # groups per chunk (2*GC psum seg banks + 2 W banks <= 8)
    BATCH_BLOCKS=48,  # gather batch cap in 128-edge blocks
    EPS=1e-5,
    TRACE=False,
)

LAST_RESULTS = None  # set by kernel() for test harness introspection
LAST_NC = None
LAST_RUN_S = None


def _ceil_div(a, b):
    return (a + b - 1) // b


def _wrap16(idx, ncols):
    """int16 idx list -> [128, ncols] tile: idx i at [i%16, i//16], replicated
    8x across the 16-partition groups (one copy per GpSimd Q7 core)."""
    n = idx.shape[0]
    assert n == ncols * 16
    w = np.ascontiguousarray(idx.reshape(ncols, 16).T)
    return np.tile(w, (8, 1))


def _balance_nodes(cfg, src, dst):
    """Assign dst nodes to (core, group) slots so that every (group-pos,
    bank) bucket has a near-equal edge count on all 8 cores.  Nodes with
    identical per-bank in-degree profiles are dealt round-robin across all
    slots.  Returns newpos[node] (position in the concatenated core
    layout)."""
    N = cfg["N"]
    C, NG, GRP, XB = cfg["NCORES"], _ceil_div(N // cfg["NCORES"], cfg["GRP"]), cfg["GRP"], cfg["XB"]
    NPC = N // C
    NS = C * NG
    last_w = NPC - (NG - 1) * GRP

    bank_e = src // XB
    prof = np.bincount(dst * 4 + bank_e, minlength=N * 4).reshape(N, 4)
    _, inv = np.unique(prof, axis=0, return_inverse=True)
    order_nodes = np.argsort(inv, kind="stable")
    cls_sorted = inv[order_nodes]
    seg = np.flatnonzero(np.diff(cls_sorted)) + 1
    seg_starts = np.concatenate([[0], seg, [N]])

    slot_of = np.empty(N, np.int64)
    ptr = 0
    for i in range(len(seg_starts) - 1):
        a, b = seg_starts[i], seg_starts[i + 1]
        m = b - a
        slot_of[order_nodes[a:b]] = (np.arange(m) + ptr) % NS
        ptr = (ptr + m) % NS
    # capacity fix: slots (k, NG-1) hold only last_w nodes
    cap = np.full(NS, GRP, np.int64)
    cap[(NG - 1) * C :] = last_w  # slot id s: gp = s // C, core = s % C
    by_slot = np.argsort(slot_of, kind="stable")
    fill = np.bincount(slot_of, minlength=NS)
    cum = np.concatenate([[0], np.cumsum(fill)])
    moved = []
    for s in range(NS):
        if fill[s] > cap[s]:
            moved.extend(by_slot[cum[s] + cap[s] : cum[s + 1]])
    if moved:
        room_slots = np.repeat(
            np.arange(NS), np.maximum(cap - fill, 0)
        )[: len(moved)]
        slot_of[np.array(moved)] = room_slots
    # final positions
    by_slot = np.argsort(slot_of, kind="stable")
    fill = np.bincount(slot_of, minlength=NS)
    assert (fill == cap).all()
    offs = np.arange(N) - np.repeat(
        np.concatenate([[0], np.cumsum(fill)])[:-1], fill
    )
    s_sorted = slot_of[by_slot]
    newpos = np.empty(N, np.int64)
    newpos[by_slot] = (s_sorted % C) * NPC + (s_sorted // C) * GRP + offs
    return newpos


def _preprocess(cfg, src, dst):
    """Bucket edges by (owner core, chunk, src bank, group) under the
    balanced node assignment; build the shared (block, group) pair schedule
    and per-core gather-index / dst-offset / per-edge-degree arrays."""
    N, E = cfg["N"], cfg["E"]
    C, NBANKS, GRP, GC = cfg["NCORES"], cfg["NBANKS"], cfg["GRP"], cfg["GC"]
    XB = cfg["XB"]
    NPC = N // C
    NG = _ceil_div(NPC, GRP)
    NCH = _ceil_div(NG, GC)
    assert XB * NBANKS == N and XB < 32768

    src = src.astype(np.int64)
    dst = dst.astype(np.int64)
    deg_out = np.bincount(src, minlength=N).astype(np.float32)
    deg_in = np.bincount(dst, minlength=N).astype(np.float32)

    newpos = _balance_nodes(cfg, src, dst)
    dstN = newpos[dst]

    owner = dstN // NPC
    g_of = (dstN % NPC) // GRP
    ch_of = g_of // GC
    gi_of = g_of - ch_of * GC
    bank = src // XB
    key = ((owner * NCH + ch_of) * NBANKS + bank) * GC + gi_of
    order = np.argsort(key, kind="stable")
    s_src = src[order]
    s_dstN = dstN[order]
    s_dstO = dst[order]
    s_key = key[order]

    nkey = C * NCH * NBANKS * GC
    counts = np.bincount(key, minlength=nkey).reshape(C, NCH, NBANKS, GC)
    cmax = counts.max(axis=0)  # [NCH, NBANKS, GC]
    R = ((cmax + 15) // 16) * 16
    for ch in range(NCH):
        ngr = min(GC, NG - ch * GC)
        R[ch, 0, :ngr] = np.maximum(R[ch, 0, :ngr], 16)  # bank-0 run exists
        R[ch, :, ngr:] = 0

    # stream layout: per (chunk, bank): runs at 16-slot granularity, stream
    # rounded up to whole 128-slot blocks
    run_off = {}
    stream_blk0 = {}
    pos = 0
    for ch in range(NCH):
        ngr = min(GC, NG - ch * GC)
        for b in range(NBANKS):
            assert pos % 128 == 0
            stream_blk0[(ch, b)] = pos // 128
            for gi in range(ngr):
                if R[ch, b, gi] > 0:
                    run_off[(ch, b, gi)] = pos
                    pos += int(R[ch, b, gi])
            pos = _ceil_div(pos, 128) * 128
    nidx_tot = pos
    nb_tot = nidx_tot // 128

    # (block, group) pair schedule + per-group chain flags
    pairs = []           # (t, g)
    group_pairs = {}     # g -> [pair indices in emission order]
    block_pairs = [[] for _ in range(nb_tot)]
    for ch in range(NCH):
        ngr = min(GC, NG - ch * GC)
        for b in range(NBANKS):
            for gi in range(ngr):
                if R[ch, b, gi] == 0:
                    continue
                g = ch * GC + gi
                r0 = run_off[(ch, b, gi)]
                r1 = r0 + int(R[ch, b, gi])
                for t in range(r0 // 128, (r1 - 1) // 128 + 1):
                    p = len(pairs)
                    pairs.append((t, g))
                    group_pairs.setdefault(g, []).append(p)
                    block_pairs[t].append(p)
    npairs = len(pairs)
    # valid slot count per stream tail block (gather is trimmed to the
    # 16-granular used length; matmuls must not read unwritten Gt rows)
    used_end_of = {}
    for ch in range(NCH):
        ngr = min(GC, NG - ch * GC)
        for b in range(NBANKS):
            ue = max(
                (
                    (run_off[(ch, b, gi)] + int(R[ch, b, gi]))
                    for gi in range(ngr)
                    if R[ch, b, gi] > 0
                ),
                default=0,
            )
            if ue > 0:
                used_end_of[(ch, b)] = ue
    blk_kk = np.full(nb_tot, 128, np.int64)
    for (ch, b), ue in used_end_of.items():
        t = (ue - 1) // 128
        if ue - t * 128 < 128:
            blk_kk[t] = ue - t * 128
    pair_info = []
    starts_set = {gp[0] for gp in group_pairs.values()}
    stops_set = {gp[-1] for gp in group_pairs.values()}
    for p, (t, g) in enumerate(pairs):
        pair_info.append((g, p in starts_set, p in stops_set, int(blk_kk[t])))
    # dedupe: a straddling run can emit two pairs (t, g) for consecutive
    # runs of the same g in different banks mapping to the same t — they
    # are distinct pairs (per-bank), which is fine for the psum chain.

    # gather batches: consecutive blocks within one (chunk, bank) stream.
    # nidx (16-granular) trims the stream-tail pad slots from the gather:
    # those slots have no (block, group) pairs, so they are never read.
    batches = []  # (bank, first_block, n_blocks, n_gather_idxs)
    for ch in range(NCH):
        ngr = min(GC, NG - ch * GC)
        for b in range(NBANKS):
            t0 = stream_blk0[(ch, b)]
            t1 = stream_blk0.get((ch, b + 1))
            if t1 is None:
                t1 = stream_blk0.get((ch + 1, 0), nb_tot)
            used_end = max(
                (
                    run_off[(ch, b, gi)] + int(R[ch, b, gi])
                    for gi in range(ngr)
                    if R[ch, b, gi] > 0
                ),
                default=t0 * 128,
            )
            rem = t1 - t0
            t = t0
            while rem > 0:
                nb = min(rem, cfg["BATCH_BLOCKS"])
                nidx = min(nb * 128, max(used_end - t * 128, 0))
                if nidx > 0:
                    batches.append((b, t, nb, nidx))
                t += nb
                rem -= nb
    # split the final batch so the tail dependency chain drains sooner
    for _ in range(2):
        if batches and batches[-1][2] >= 4:
            b, t, nb, nidx = batches.pop()
            nb1 = nb // 2
            n1 = min(nb1 * 128, nidx)
            batches.append((b, t, nb1, n1))
            if nidx > n1:
                batches.append((b, t + nb1, nb - nb1, nidx - n1))

    # per (k, ch, b, gi) boundaries in the sorted edge stream
    bkeys = np.arange(nkey)
    bstarts = np.searchsorted(s_key, bkeys).reshape(C, NCH, NBANKS, GC)
    bends = np.searchsorted(s_key, bkeys, side="right").reshape(C, NCH, NBANKS, GC)

    gidx_cores = []
    doff_cores = []
    dgo_cores = []
    dgi_cores = []
    for k in range(C):
        gidx = np.zeros(nidx_tot, np.int16)
        dmod = np.full(nidx_tot, -1.0e6, np.float32)
        dgo = np.ones(nidx_tot, np.float32)
        dgi = np.ones(nidx_tot, np.float32)
        for (ch, b, gi), p0 in run_off.items():
            s, e = int(bstarts[k, ch, b, gi]), int(bends[k, ch, b, gi])
            cnt = e - s
            if cnt == 0:
                continue
            gidx[p0 : p0 + cnt] = (s_src[s:e] % XB).astype(np.int16)
            dmod[p0 : p0 + cnt] = (s_dstN[s:e] % NPC).astype(np.float32)
            dgo[p0 : p0 + cnt] = deg_out[s_src[s:e]]
            dgi[p0 : p0 + cnt] = deg_in[s_dstO[s:e]]
        dmod2 = dmod.reshape(nb_tot, 128)
        doff = np.empty((npairs, 128), np.float32)
        for p, (t, g) in enumerate(pairs):
            doff[p] = dmod2[t] - np.float32(g * GRP)
        gidx_cores.append(_wrap16(gidx, nidx_tot // 16))
        doff_cores.append(np.ascontiguousarray(doff.T))
        dgo_cores.append(np.ascontiguousarray(dgo.reshape(nb_tot, 128).T))
        dgi_cores.append(np.ascontiguousarray(dgi.reshape(nb_tot, 128).T))

    meta = dict(
        NPC=NPC,
        NG=NG,
        nidx_tot=nidx_tot,
        nb_tot=nb_tot,
        npairs=npairs,
        pairs=pairs,
        pair_info=pair_info,
        block_pairs=block_pairs,
        batches=batches,
        newpos=newpos,
    )
    return meta, gidx_cores, doff_cores, dgo_cores, dgi_cores


def _build_nc(cfg, meta):
    N, IN, OUT, C = cfg["N"], cfg["IN"], cfg["OUT"], cfg["NCORES"]
    GRP, NBANKS, XB, GC = cfg["GRP"], cfg["NBANKS"], cfg["XB"], cfg["GC"]
    NPC, NG = meta["NPC"], meta["NG"]
    nidx_tot, nb_tot = meta["nidx_tot"], meta["nb_tot"]
    npairs = meta["npairs"]
    pair_info = meta["pair_info"]
    block_pairs = meta["block_pairs"]
    batches = meta["batches"]
    XK = _ceil_div(IN, 128)
    assert OUT == 128 and GRP == 128 and IN == 256
    last_w = NPC - (NG - 1) * GRP  # valid dst cols in the last group

    nc = bacc.Bacc(
        "TRN2", target_bir_lowering=False, debug=False, num_devices=C
    )

    # ---- external inputs ----
    xb = [
        nc.dram_tensor(f"xb{q}", [XB, IN], BF16, kind="ExternalInput")
        for q in range(NBANKS)
    ]
    wt = [
        nc.dram_tensor(f"wt{j}", [128, OUT], BF16, kind="ExternalInput")
        for j in range(XK)
    ]
    gidx_d = nc.dram_tensor("gidx", [128, nidx_tot // 16], I16, kind="ExternalInput")
    doff_d = nc.dram_tensor("doff", [128, npairs], F32, kind="ExternalInput")
    dgo_d = nc.dram_tensor("dgo", [128, nb_tot], BF16, kind="ExternalInput")
    dgi_d = nc.dram_tensor("dgi", [128, nb_tot], BF16, kind="ExternalInput")
    iota_d = nc.dram_tensor("iota", [128, GRP], BF16, kind="ExternalInput")
    btc_d = nc.dram_tensor("btc", [OUT, 1], F32, kind="ExternalInput")
    gmc_d = nc.dram_tensor("gmc", [OUT, 1], F32, kind="ExternalInput")
    bbc_d = nc.dram_tensor("bbc", [OUT, 1], F32, kind="ExternalInput")

    ypadT_d = nc.dram_tensor("ypadT", [OUT, NG * GRP], BF16, kind="ExternalOutput")

    bmax = max(nb for _, _, nb, _ in batches)

    with tile.TileContext(nc) as tc:
        with (
            tc.tile_pool(name="const", bufs=1) as cpool,
            tc.tile_pool(name="dram", bufs=1, space="DRAM") as dpool,
            tc.tile_pool(name="agg", bufs=1) as apool,
            tc.tile_pool(name="gath", bufs=6) as gpool,
            tc.tile_pool(name="mpool", bufs=10) as mpool,
            tc.tile_pool(name="asb", bufs=3) as asbp,
            tc.tile_pool(name="sq", bufs=3) as sqp,
            tc.tile_pool(name="pseg", bufs=1, space="PSUM") as psegp,
            tc.tile_pool(name="pw", bufs=2, space="PSUM") as pwp,
        ):
            # ---- constants / small tiles ----
            iota_t = cpool.tile([128, GRP], BF16)
            btc_t = cpool.tile([OUT, 1], F32)
            gmc_t = cpool.tile([OUT, 1], F32)
            bbc_t = cpool.tile([OUT, 1], F32)
            gidx_t = cpool.tile([128, nidx_tot // 16], I16)
            doff_t = cpool.tile([128, npairs], F32)
            dgo_t = cpool.tile([128, nb_tot], BF16)
            dgi_t = cpool.tile([128, nb_tot], BF16)
            s_t = cpool.tile([128, nb_tot], F32)
            r2_t = cpool.tile([128, nb_tot], F32)
            stats_s = cpool.tile([OUT, NG], F32)
            stats_q = cpool.tile([OUT, NG], F32)
            wts = []
            for j in range(XK):
                wtile = cpool.tile([128, OUT], BF16, name=f"wt_s{j}")
                nc.sync.dma_start(wtile[:], wt[j][:])
                wts.append(wtile)

            # split the big index load so the first gather batches and the
            # first M-gens don't wait for the whole-tensor DMAs
            gcols = nidx_tot // 16
            gsplits = [0, min(256, gcols), min(2048, gcols), gcols]
            for a, z in zip(gsplits, gsplits[1:]):
                if z > a:
                    nc.sync.dma_start(gidx_t[:, a:z], gidx_d[:, a:z])
            nc.sync.dma_start(iota_t[:], iota_d[:])
            bsplit = [0, min(256, nb_tot), nb_tot]
            for a, z in zip(bsplit, bsplit[1:]):
                if z > a:
                    nc.sync.dma_start(dgo_t[:, a:z], dgo_d[:, a:z])
                    nc.sync.dma_start(dgi_t[:, a:z], dgi_d[:, a:z])
            psplit = [0, min(256, npairs), npairs]
            for a, z in zip(psplit, psplit[1:]):
                if z > a:
                    nc.sync.dma_start(doff_t[:, a:z], doff_d[:, a:z])
            nc.sync.dma_start(btc_t[:], btc_d[:])
            nc.sync.dma_start(gmc_t[:], gmc_d[:])
            nc.sync.dma_start(bbc_t[:], bbc_d[:])

            # per-edge norm scale s = rsqrt(max(dgo,1)) * rsqrt(max(dgi,1))
            # in two column segments so early blocks unblock fast
            for a, z in zip(bsplit, bsplit[1:]):
                if z <= a:
                    continue
                for deg_t, out_t in ((dgo_t, s_t), (dgi_t, r2_t)):
                    nc.vector.tensor_scalar(
                        out_t[:, a:z], deg_t[:, a:z], 1.0, None,
                        op0=mybir.AluOpType.max,
                    )
                    nc.vector.reciprocal(out_t[:, a:z], out_t[:, a:z])
                    nc.scalar.activation(
                        out_t[:, a:z], out_t[:, a:z],
                        mybir.ActivationFunctionType.Sqrt,
                    )
                nc.vector.tensor_mul(s_t[:, a:z], s_t[:, a:z], r2_t[:, a:z])

            # h table (feature-major, bf16): agg_t[:, g, d] = h[o, g*128+d]
            agg_t = apool.tile([OUT, NG, GRP], BF16)
            # zero the last group's pad columns (stats square-pass reads them)
            nc.gpsimd.memset(agg_t[:, NG - 1, :], 0.0)

            # internal DRAM for the BN-stats collective (AllGather is ~2x
            # cheaper than AllReduce in fixed cost; reduce locally instead)
            stats_in = dpool.tile([OUT, 2], F32)
            stats_out = dpool.tile([C * OUT, 2], F32, addr_space="Shared")

            # ---- main loop: gather + one-hot matmul segmented sum ----
            cur_ps = {}  # gi -> (psA, psB)
            for bank, t0, nblk, nidx in batches:
                Gt = gpool.tile([128, bmax, IN], BF16, tag="G")
                nc.gpsimd.dma_gather(
                    Gt[:, : _ceil_div(nidx, 128), :],
                    xb[bank][:],
                    gidx_t[:, t0 * 8 : t0 * 8 + nidx // 16],
                    nidx,
                    nidx,
                    IN,
                    single_packet=False,
                )
                for j in range(nblk):
                    t = t0 + j
                    for p in block_pairs[t]:
                        g, is_start, is_stop, kk = pair_info[p]
                        gi = g % GC
                        Mt = mpool.tile([128, GRP], BF16, tag="M")
                        nc.vector.tensor_scalar(
                            Mt[:],
                            iota_t[:],
                            doff_t[:, p : p + 1],
                            s_t[:, t : t + 1],
                            op0=mybir.AluOpType.is_equal,
                            op1=mybir.AluOpType.mult,
                        )
                        if is_start:
                            psA = psegp.tile(
                                [128, GRP], F32, tag=f"sA{gi}", name=f"psA{gi}"
                            )
                            psB = psegp.tile(
                                [128, GRP], F32, tag=f"sB{gi}", name=f"psB{gi}"
                            )
                            cur_ps[gi] = (psA, psB)
                        psA, psB = cur_ps[gi]
                        nc.tensor.matmul(
                            psA[:], Gt[:kk, j, 0:128], Mt[:kk, :],
                            start=is_start, stop=is_stop,
                        )
                        nc.tensor.matmul(
                            psB[:], Gt[:kk, j, 128:256], Mt[:kk, :],
                            start=is_start, stop=is_stop,
                        )
                        if not is_stop:
                            continue
                        aggA = asbp.tile([128, GRP], BF16, tag="aggA")
                        aggB = asbp.tile([128, GRP], BF16, tag="aggB")
                        nc.scalar.activation(
                            aggA[:], psA[:], mybir.ActivationFunctionType.Copy
                        )
                        nc.scalar.activation(
                            aggB[:], psB[:], mybir.ActivationFunctionType.Copy
                        )
                        pso = pwp.tile([OUT, GRP], F32, tag="w")
                        nc.tensor.matmul(
                            pso[:], wts[0][:], aggA[:], start=True, stop=False
                        )
                        nc.tensor.matmul(
                            pso[:], wts[1][:], aggB[:], start=False, stop=True
                        )
                        w = GRP if g < NG - 1 else last_w
                        nc.scalar.activation(
                            agg_t[:, g, :w],
                            pso[:, :w],
                            mybir.ActivationFunctionType.Relu,
                            bias=btc_t[:, 0:1],
                            accum_out=stats_s[:, g : g + 1],
                        )
                        sqt = sqp.tile([OUT, GRP], BF16, tag="sq")
                        nc.scalar.activation(
                            sqt[:, :w],
                            agg_t[:, g, :w],
                            mybir.ActivationFunctionType.Square,
                            accum_out=stats_q[:, g : g + 1],
                        )

            # ---- BN stats AllReduce + affine finalize ----
            stsb = cpool.tile([OUT, 2], F32)
            nc.vector.tensor_reduce(
                stsb[:, 0:1], stats_s[:], mybir.AxisListType.X, mybir.AluOpType.add
            )
            nc.vector.tensor_reduce(
                stsb[:, 1:2], stats_q[:], mybir.AxisListType.X, mybir.AluOpType.add
            )
            nc.sync.dma_start(stats_in[:], stsb[:])
            nc.gpsimd.collective_compute(
                "AllGather",
                mybir.AluOpType.bypass,
                replica_groups=[list(range(C))],
                ins=[stats_in[:]],
                outs=[stats_out[:]],
            )
            stall = cpool.tile([OUT, C, 2], F32)
            nc.sync.dma_start(
                stall[:], stats_out[:].rearrange("(c p) f -> p c f", c=C)
            )
            strb = cpool.tile([OUT, 2], F32)
            nc.vector.tensor_reduce(
                strb[:],
                stall[:].rearrange("p c f -> p f c"),
                mybir.AxisListType.X,
                mybir.AluOpType.add,
            )

            mu = cpool.tile([OUT, 1], F32)
            ex2 = cpool.tile([OUT, 1], F32)
            var = cpool.tile([OUT, 1], F32)
            S_t = cpool.tile([OUT, 1], F32)
            T_t = cpool.tile([OUT, 1], F32)
            inv_n = 1.0 / float(N)
            nc.scalar.activation(
                mu[:], strb[:, 0:1], mybir.ActivationFunctionType.Copy, scale=inv_n
            )
            nc.scalar.activation(
                ex2[:], strb[:, 1:2], mybir.ActivationFunctionType.Copy, scale=inv_n
            )
            nc.scalar.activation(var[:], mu[:], mybir.ActivationFunctionType.Square)
            nc.vector.tensor_sub(var[:], ex2[:], var[:])
            # var <- rsqrt(var + eps) (ACT Rsqrt is banned for accuracy)
            nc.scalar.activation(
                var[:], var[:], mybir.ActivationFunctionType.Copy,
                bias=float(cfg["EPS"]),
            )
            nc.vector.reciprocal(var[:], var[:])
            nc.scalar.activation(var[:], var[:], mybir.ActivationFunctionType.Sqrt)
            nc.vector.tensor_mul(S_t[:], gmc_t[:], var[:])
            nc.vector.tensor_mul(T_t[:], mu[:], S_t[:])
            nc.vector.tensor_sub(T_t[:], bbc_t[:], T_t[:])

            # ---- apply affine: y = h*S + T (ACT/DVE quarters, DMA
            # interleaved so the writeback overlaps the affine) ----
            ypadT_view = ypadT_d[:].rearrange("p (g f) -> p g f", g=NG)
            qs = [0, NG // 4, NG // 2, 3 * NG // 4, NG]
            for qi in range(4):
                a, z = qs[qi], qs[qi + 1]
                if qi % 2 == 0:
                    nc.scalar.activation(
                        agg_t[:, a:z, :],
                        agg_t[:, a:z, :],
                        mybir.ActivationFunctionType.Identity,
                        bias=T_t[:, 0:1],
                        scale=S_t[:, 0:1],
                    )
                else:
                    nc.vector.tensor_scalar(
                        agg_t[:, a:z, :],
                        agg_t[:, a:z, :],
                        S_t[:, 0:1],
                        T_t[:, 0:1],
                        op0=mybir.AluOpType.mult,
                        op1=mybir.AluOpType.add,
                    )
                nc.sync.dma_start(ypadT_view[:, a:z, :], agg_t[:, a:z, :])

    nc.compile()
    return nc


def kernel(x, src, dst, W, b, gamma, beta):
    global LAST_RESULTS
    cfg = CFG
    N, E, IN, OUT, C = cfg["N"], cfg["E"], cfg["IN"], cfg["OUT"], cfg["NCORES"]
    GRP, XB, NBANKS = cfg["GRP"], cfg["XB"], cfg["NBANKS"]
    assert x.shape == (N, IN) and W.shape == (IN, OUT)
    assert src.shape == (E,) and dst.shape == (E,)

    meta, gidx_cores, doff_cores, dgo_cores, dgi_cores = _preprocess(cfg, src, dst)
    NPC, NG = meta["NPC"], meta["NG"]
    newpos = meta["newpos"]
    XK = _ceil_div(IN, 128)

    nc = _build_nc(cfg, meta)

    import ml_dtypes

    xbf = np.asarray(x, np.float32).astype(ml_dtypes.bfloat16)  # [N, IN]
    Wn = np.asarray(W, np.float32)

    iota = np.tile(
        np.arange(GRP, dtype=np.float32)[None, :], (128, 1)
    ).astype(ml_dtypes.bfloat16)
    btc = np.ascontiguousarray(np.asarray(b, np.float32)[:, None])
    gmc = np.ascontiguousarray(np.asarray(gamma, np.float32)[:, None])
    bbc = np.ascontiguousarray(np.asarray(beta, np.float32)[:, None])

    xbanks = {
        f"xb{q}": np.ascontiguousarray(xbf[q * XB : (q + 1) * XB, :])
        for q in range(NBANKS)
    }
    wmap = {
        f"wt{j}": np.ascontiguousarray(
            Wn[j * 128 : (j + 1) * 128, :]
        ).astype(ml_dtypes.bfloat16)
        for j in range(XK)
    }

    in_maps = []
    for k in range(C):
        im = {
            "gidx": gidx_cores[k],
            "doff": doff_cores[k],
            "dgo": dgo_cores[k].astype(ml_dtypes.bfloat16),
            "dgi": dgi_cores[k].astype(ml_dtypes.bfloat16),
            "iota": iota,
            "btc": btc,
            "gmc": gmc,
            "bbc": bbc,
        }
        im.update(xbanks)
        im.update(wmap)
        in_maps.append(im)

    if cfg.get("SIM"):
        from concourse.bass_interp import MultiCoreSim

        sim = MultiCoreSim(nc, num_cores=C)
        for k, core_sim in sim.cores.items():
            for name, val in in_maps[k].items():
                core_sim.tensor(name)[:] = val
        sim.simulate()
        ycomp = np.empty((N, OUT), np.float32)
        for k in range(C):
            ycomp[k * NPC : (k + 1) * NPC] = (
                sim.cores[k].tensor("ypadT")[:, :NPC].astype(np.float32).T
            )
        return ycomp[newpos]

    global LAST_NC, LAST_RUN_S
    LAST_NC = nc
    import time as _time

    _t0 = _time.time()
    res = bass_utils.run_bass_kernel_spmd(
        nc,
        in_maps,
        core_ids=list(range(C)),
        trace=cfg.get("TRACE", False),
    )
    LAST_RUN_S = _time.time() - _t0
    LAST_RESULTS = res

    ycomp = np.empty((N, OUT), np.float32)
    for k in range(C):
        ycomp[k * NPC : (k + 1) * NPC] = (
            res.results[k]["ypadT"][:, :NPC].astype(np.float32).T
        )
    return ycomp[newpos]
